# revision 10
# baseline (speedup 1.0000x reference)
# kernel.py — 3x3 avg-pool (stride 1, VALID) over NCHW f32 via Bass/Tile on 8 TRN2 cores.
#
# Layout: one image per SBUF partition, [64, 64] in the free dim.  Both pooling
# directions are free-dim shifted adds (separable 3-tap x 3-tap):
#   H-pass: mid[r, w] = x[r,w] + x[r,w+1] + x[r,w+2]       (2 adds)
#   V-pass: sum[r, w] = mid[r,w] + mid[r+1,w] + mid[r+2,w] (2 adds)
#   scale:  out = sum * (1/9)                              (ScalarE, in-place)
# The adds are fp32 tensor_tensor (1x on DVE), so the work is row-split between
# DVE (out rows 0..VD-1) and GPSIMD (out rows VD..61).  The two boundary mid
# rows are computed by both engines so the chains stay fully independent.
# fp32 TT never enters a 2-port DVE perf mode, so DVE and GPSIMD don't contend
# for the shared SBUF port pair; the scale runs on ScalarE (own ports).
# DMA-touched pools use bufs=NT (no slot reuse) because a DMA instruction can
# carry at most one sync-wait and Tile doesn't elide transitively-implied ones.
# DMA descriptors are contiguous per partition (16 KB loads / 9.9+5.5 KB stores).
#
# Full input (16, 256, 64, 64) is sharded 4096 images -> 8 cores x 512 images.

import numpy as np

N_CORES = 8
N, C = 16, 256
H = W = 64
OH = OW = 62
P = 128                        # SBUF partitions = images per mega-tile
IMGS_PER_CORE = (N * C) // N_CORES    # 512
NT = IMGS_PER_CORE // P        # 4 mega-tiles per core

VD = 40                        # output rows handled by DVE; rest go to GPSIMD

_nc_cache = {}


def _split_multiwait(nc, max_waits=1):
    """Walrus's codegen allows only one embedded sync-wait per instruction
    (HW-decode struct limit); Tile's kernel-tail drain carries the whole
    global clock.  Move excess waits onto single-wait EventSemaphore
    instructions inserted immediately before the offending instruction on
    the same engine."""
    import concourse.mybir as mb

    for f in nc.m.functions:
        for b in f.blocks:
            new_list = []
            for inst in b.instructions:
                si = getattr(inst, "sync_info", None)
                if si is not None and len(si.on_wait) > max_waits:
                    waits = list(si.on_wait)
                    extra, keep = waits[:-max_waits], waits[-max_waits:]
                    for k, w in enumerate(extra):
                        es = mb.InstEventSemaphore(
                            name=f"{inst.name}-esw{k}", ins=[], outs=[],
                            engine=inst.engine)
                        es.sync_info = mb.SyncInfo(on_wait=[w], on_update=[])
                        nc.register_instruction(es)
                        new_list.append(es)
                    inst.sync_info = mb.SyncInfo(
                        on_wait=keep, on_update=list(si.on_update))
                new_list.append(inst)
            b.instructions[:] = new_list


def _build_nc(vd=VD):
    import concourse.bass as bass
    import concourse.mybir as mybir
    from concourse.tile import TileContext

    f32 = mybir.dt.float32

    nc = bass.Bass()
    x = nc.declare_dram_parameter("x", [IMGS_PER_CORE, H, W], f32, isOutput=False)
    o = nc.declare_dram_parameter("o", [IMGS_PER_CORE, OH, OW], f32, isOutput=True)

    hd = vd + 2        # mid rows 0..hd-1 computed by DVE
    hg = H - vd        # mid rows vd..63 computed by GPSIMD
    vg = OH - vd       # output rows vd..61 computed by GPSIMD

    with TileContext(nc) as tc:
        with (
            tc.tile_pool(name="xp", bufs=NT) as xp,
            tc.tile_pool(name="md", bufs=NT) as md,
            tc.tile_pool(name="mg", bufs=NT) as mg,
            tc.tile_pool(name="op", bufs=NT) as op,
        ):
            for t in range(NT):
                xt = xp.tile([P, H, W], f32)
                nc.sync.dma_start(out=xt[:], in_=x[t * P:(t + 1) * P])

                # ---- DVE chain: out rows 0..vd-1 ----
                mid_d = md.tile([P, hd, OW], f32)
                nc.vector.tensor_add(
                    out=mid_d[:], in0=xt[:, 0:hd, 0:62], in1=xt[:, 0:hd, 1:63])
                nc.vector.tensor_add(
                    out=mid_d[:], in0=mid_d[:], in1=xt[:, 0:hd, 2:64])
                # ---- GPSIMD chain: out rows vd..61 ----
                mid_g = mg.tile([P, hg, OW], f32)
                nc.gpsimd.tensor_add(
                    out=mid_g[:], in0=xt[:, vd:H, 0:62], in1=xt[:, vd:H, 1:63])
                nc.gpsimd.tensor_add(
                    out=mid_g[:], in0=mid_g[:], in1=xt[:, vd:H, 2:64])

                ot = op.tile([P, OH, OW], f32)
                nc.vector.tensor_add(
                    out=ot[:, 0:vd, :], in0=mid_d[:, 0:vd, :], in1=mid_d[:, 1:vd + 1, :])
                nc.vector.tensor_add(
                    out=ot[:, 0:vd, :], in0=ot[:, 0:vd, :], in1=mid_d[:, 2:vd + 2, :])
                nc.gpsimd.tensor_add(
                    out=ot[:, vd:OH, :], in0=mid_g[:, 0:vg, :], in1=mid_g[:, 1:vg + 1, :])
                nc.gpsimd.tensor_add(
                    out=ot[:, vd:OH, :], in0=ot[:, vd:OH, :], in1=mid_g[:, 2:vg + 2, :])

                # ---- 1/9 on ScalarE (own SBUF ports; never contends) ----
                nc.scalar.mul(out=ot[:, 0:vd, :], in_=ot[:, 0:vd, :], mul=1.0 / 9.0)
                nc.scalar.mul(out=ot[:, vd:OH, :], in_=ot[:, vd:OH, :], mul=1.0 / 9.0)

                nc.sync.dma_start(out=o[t * P:(t + 1) * P], in_=ot[:])

    _split_multiwait(nc)
    nc.finalize()
    return nc


def _get_nc(vd=VD):
    if vd not in _nc_cache:
        _nc_cache[vd] = _build_nc(vd)
    return _nc_cache[vd]


def run(x, trace=False, vd=VD, **spmd_kwargs):
    """Run the pool kernel on 8 cores. x: (16,256,64,64) f32. Returns
    (output (16,256,62,62) f32, BassKernelResults)."""
    from concourse.bass_utils import run_bass_kernel_spmd

    x = np.ascontiguousarray(np.asarray(x, dtype=np.float32))
    assert x.shape == (N, C, H, W), x.shape
    shards = x.reshape(N_CORES, IMGS_PER_CORE, H, W)
    in_maps = [{"x": shards[c]} for c in range(N_CORES)]
    nc = _get_nc(vd)
    res = run_bass_kernel_spmd(
        nc, in_maps, list(range(N_CORES)), trace=trace, **spmd_kwargs
    )
    out = np.stack([res.results[c]["o"] for c in range(N_CORES)], axis=0)
    return out.reshape(N, C, OH, OW), res


def kernel(x):
    out, _ = run(x, trace=False)
    return out


# revision 11
# speedup vs baseline: 1.0981x; 1.0981x over previous
# kernel.py — 3x3 avg-pool (stride 1, VALID) over NCHW f32 via Bass/Tile on 8 TRN2 cores.
#
# Layout: one image per SBUF partition, [64, 64] in the free dim.  Both pooling
# directions are then free-dim shifted adds (separable 3-tap x 3-tap):
#   H-pass: mid[r, w] = x[r,w] + x[r,w+1] + x[r,w+2]       (2 adds)
#   V-pass: sum[r, w] = mid[r,w] + mid[r+1,w] + mid[r+2,w] (2 adds)
#   scale:  out = sum * (1/9)                              (ScalarE, in-place)
# The adds are fp32 tensor_tensor (1x mode), row-split between DVE (out rows
# 0..VD-1) and GPSIMD (out rows VD..61); the two boundary mid rows are computed
# by both engines so the chains are fully independent.  fp32 TT never enters a
# 2-port DVE perf mode, so DVE and GPSIMD don't contend for the shared SBUF
# port pair; the 1/9 runs on ScalarE (own ports).  The PE is useless here:
# fp32 matmul is 4 cyc/row and HW-inexact (probed), fp32r is ~12-bit.
#
# DMA: flat layout gives perfectly contiguous per-partition descriptors
# (16 KB loads, 9.7/5.5 KB stores).  Pools use bufs=NT (no slot reuse) so
# DMAs need at most one sync-wait; remaining multi-wait instructions (the
# Tile kernel-tail drain) are legalized by _split_multiwait, since walrus
# codegen allows only one embedded sync-wait per instruction.  Tile 0's load
# is split so compute starts earlier; tile 3's store is split across the two
# HWDGE rings (SP + ACT) to shorten the tail.
#
# Full input (16, 256, 64, 64) is sharded 4096 images -> 8 cores x 512 images
# (contiguous N*C ranges), no cross-core communication.

import numpy as np

N_CORES = 8
N, C = 16, 256
H = W = 64
OH = OW = 62
P = 128                        # SBUF partitions = images per mega-tile
IMGS_PER_CORE = (N * C) // N_CORES    # 512
NT = IMGS_PER_CORE // P        # 4 mega-tiles per core

VD = 39                        # output rows handled by DVE; rest go to GPSIMD

_nc_cache = {}


def _split_multiwait(nc, max_waits=1):
    """Walrus's codegen allows only one embedded sync-wait per instruction
    (HW-decode struct limit); Tile's kernel-tail drain carries the whole
    global clock.  Move excess waits onto single-wait EventSemaphore
    instructions inserted immediately before the offending instruction on
    the same engine."""
    import concourse.mybir as mb

    for f in nc.m.functions:
        for b in f.blocks:
            new_list = []
            for inst in b.instructions:
                si = getattr(inst, "sync_info", None)
                if si is not None and len(si.on_wait) > max_waits:
                    waits = list(si.on_wait)
                    extra, keep = waits[:-max_waits], waits[-max_waits:]
                    for k, w in enumerate(extra):
                        es = mb.InstEventSemaphore(
                            name=f"{inst.name}-esw{k}", ins=[], outs=[],
                            engine=inst.engine)
                        es.sync_info = mb.SyncInfo(on_wait=[w], on_update=[])
                        nc.register_instruction(es)
                        new_list.append(es)
                    inst.sync_info = mb.SyncInfo(
                        on_wait=keep, on_update=list(si.on_update))
                new_list.append(inst)
            b.instructions[:] = new_list


def _build_nc(vd=VD):
    import concourse.bass as bass
    import concourse.mybir as mybir
    from concourse.tile import TileContext

    f32 = mybir.dt.float32

    nc = bass.Bass()
    x = nc.declare_dram_parameter("x", [IMGS_PER_CORE, H, W], f32, isOutput=False)
    o = nc.declare_dram_parameter("o", [IMGS_PER_CORE, OH, OW], f32, isOutput=True)

    hd = vd + 2        # mid rows 0..hd-1 computed by DVE
    hg = H - vd        # mid rows vd..63 computed by GPSIMD
    vg = OH - vd       # output rows vd..61 computed by GPSIMD

    with TileContext(nc) as tc:
        with (
            tc.tile_pool(name="xp", bufs=NT) as xp,
            tc.tile_pool(name="md", bufs=NT) as md,
            tc.tile_pool(name="mg", bufs=NT) as mg,
            tc.tile_pool(name="op", bufs=NT) as op,
        ):
            for t in range(NT):
                xt = xp.tile([P, H, W], f32)
                if t == 0:
                    # split first load: DVE's H-pass rows arrive ~3 us sooner
                    nc.sync.dma_start(out=xt[:, 0:hd, :], in_=x[0:P, 0:hd])
                    nc.sync.dma_start(out=xt[:, hd:H, :], in_=x[0:P, hd:H])
                else:
                    nc.sync.dma_start(out=xt[:], in_=x[t * P:(t + 1) * P])

                # ---- DVE chain: out rows 0..vd-1 ----
                mid_d = md.tile([P, hd, OW], f32)
                nc.vector.tensor_add(
                    out=mid_d[:], in0=xt[:, 0:hd, 0:62], in1=xt[:, 0:hd, 1:63])
                nc.vector.tensor_add(
                    out=mid_d[:], in0=mid_d[:], in1=xt[:, 0:hd, 2:64])
                # ---- GPSIMD chain: out rows vd..61 ----
                mid_g = mg.tile([P, hg, OW], f32)
                nc.gpsimd.tensor_add(
                    out=mid_g[:], in0=xt[:, vd:H, 0:62], in1=xt[:, vd:H, 1:63])
                nc.gpsimd.tensor_add(
                    out=mid_g[:], in0=mid_g[:], in1=xt[:, vd:H, 2:64])

                ot = op.tile([P, OH, OW], f32)
                nc.vector.tensor_add(
                    out=ot[:, 0:vd, :], in0=mid_d[:, 0:vd, :], in1=mid_d[:, 1:vd + 1, :])
                nc.vector.tensor_add(
                    out=ot[:, 0:vd, :], in0=ot[:, 0:vd, :], in1=mid_d[:, 2:vd + 2, :])
                nc.gpsimd.tensor_add(
                    out=ot[:, vd:OH, :], in0=mid_g[:, 0:vg, :], in1=mid_g[:, 1:vg + 1, :])
                nc.gpsimd.tensor_add(
                    out=ot[:, vd:OH, :], in0=ot[:, vd:OH, :], in1=mid_g[:, 2:vg + 2, :])

                # ---- 1/9 on ScalarE (own SBUF ports; never contends) ----
                nc.scalar.mul(out=ot[:, 0:vd, :], in_=ot[:, 0:vd, :], mul=1.0 / 9.0)
                nc.scalar.mul(out=ot[:, vd:OH, :], in_=ot[:, vd:OH, :], mul=1.0 / 9.0)

                if t == NT - 1:
                    # split last store across both HWDGE rings: shorter tail
                    nc.sync.dma_start(
                        out=o[t * P:(t + 1) * P, 0:vd, :], in_=ot[:, 0:vd, :])
                    nc.scalar.dma_start(
                        out=o[t * P:(t + 1) * P, vd:OH, :], in_=ot[:, vd:OH, :])
                else:
                    nc.sync.dma_start(out=o[t * P:(t + 1) * P], in_=ot[:])

    _split_multiwait(nc)
    nc.finalize()
    return nc


def _get_nc(vd=VD):
    if vd not in _nc_cache:
        _nc_cache[vd] = _build_nc(vd)
    return _nc_cache[vd]


def run(x, trace=False, vd=VD, **spmd_kwargs):
    """Run the pool kernel on 8 cores. x: (16,256,64,64) f32. Returns
    (output (16,256,62,62) f32, BassKernelResults)."""
    from concourse.bass_utils import run_bass_kernel_spmd

    x = np.ascontiguousarray(np.asarray(x, dtype=np.float32))
    assert x.shape == (N, C, H, W), x.shape
    shards = x.reshape(N_CORES, IMGS_PER_CORE, H, W)
    in_maps = [{"x": shards[c]} for c in range(N_CORES)]
    nc = _get_nc(vd)
    res = run_bass_kernel_spmd(
        nc, in_maps, list(range(N_CORES)), trace=trace, **spmd_kwargs
    )
    out = np.stack([res.results[c]["o"] for c in range(N_CORES)], axis=0)
    return out.reshape(N, C, OH, OW), res


def kernel(x):
    out, _ = run(x, trace=False)
    return out
